# revision 4
# baseline (speedup 1.0000x reference)
"""Trainium2 Bass kernel for nn_ChoopyLoss (F1@k weighted loss).

Math: for each row i,
    cum[i,j] = prefix-sum of labels[i, :j+1]
    T[i]     = cum[i, N-1]  (total relevant)
    f1[i,j]  = 2*cum[i,j] / (j+1 + T[i])     # algebraic simplification of
                                             # 2*prec*rec/(prec+rec); holds
                                             # for cum==0 and T==0 too
    loss = -(1/B) * sum_{i,j} output[i,j] * f1[i,j]

Sharding: pure data parallel over batch. Each of the 8 cores gets 256 rows,
computes per-partition partial sums, host sums the 8x[128,16] partials.

Per-core pipeline (2 row-blocks of 128 rows x 8192):
  DMA  : labels(int32) + output(f32) full-row loads
  DVE  : tensor_tensor_scan (cumsum) -> bf16; reciprocal_approx_fast;
         bf16 tensor_mul; tensor_tensor_reduce (fused mul+sum, scale=-2/B)
  ACT  : f32->bf16 cast of output; d = k + T (iota int16 + per-row bias)
  POOL : f32->bf16 cast of reciprocals; iota (one-time)
"""

import numpy as np

import concourse.bass as bass
import concourse.mybir as mybir
from concourse.bass_utils import run_bass_kernel_spmd
from concourse.tile import TileContext
from concourse.vector_clock import ScopedClock

B, N = 2048, 8192
NCORES = 8
ROWS_PER_CORE = B // NCORES          # 256
P = 128
RB = ROWS_PER_CORE // P              # 2 row-blocks
W = 1024                             # chunk width for pointwise stages
NCH = N // W

f32 = mybir.dt.float32
bf16 = mybir.dt.bfloat16
i32 = mybir.dt.int32
i16 = mybir.dt.int16
Alu = mybir.AluOpType
Act = mybir.ActivationFunctionType


MAX_WAITS = 1  # this walrus build rejects >MAX_WAITS sync waits per instruction


def _split_sync_waits(nc, max_waits=MAX_WAITS):
    """Walrus here rejects instructions carrying many sem waits ("Too many
    sync wait commands"). Hoist excess waits onto same-engine NoOps placed
    immediately before the instruction."""
    import bass_rust

    for f in nc.m.functions:
        for bb in f.blocks:
            new_insts = []
            for inst in bb.instructions:
                si = inst.sync_info
                waits = list(si.on_wait) if si and si.on_wait else []
                if len(waits) > max_waits:
                    keep = waits[:max_waits]
                    extra = waits[max_waits:]
                    for i in range(0, len(extra), max_waits):
                        nop = bass_rust.InstNoOp(
                            name=nc.get_next_instruction_name(), ins=[], outs=[]
                        )
                        nop.engine = inst.engine
                        nop.sync_info = mybir.SyncInfo(
                            on_wait=extra[i : i + max_waits], on_update=[]
                        )
                        nc.register_instruction(nop, overwrite=True)
                        new_insts.append(nop)
                    si.on_wait = keep
                new_insts.append(inst)
            bb.instructions[:] = new_insts


class TileContextSplitDrain(TileContext):
    """Stock TileContext puts one wait per live proc on the kernel-tail
    drain; pre-split those, then run the generic wait-splitter on exit."""

    def _drain_and_barrier(self, tick_clock, wait_clock):
        nop = self.nc.sync.nop(nofuse=True, hint="pre_drain_waits")
        wait_clock.add_sem_waits(
            nop.ins, ScopedClock({None: tick_clock.global_clock})
        )
        si = nop.ins.sync_info
        waits = list(si.on_wait or []) if si else []
        if si:
            si.on_wait = waits[:1]
        for w in waits[1:]:
            n2 = self.nc.sync.nop(nofuse=True, hint="pre_drain_waits")
            n2.ins.sync_info = mybir.SyncInfo(on_wait=[w], on_update=[])

        self.nc.sync.drain()
        self.nc.all_engine_barrier()
        assert self.sems is not None
        popped = self.nc._tile_sem_poison_stack.pop()
        assert popped is self._sem_poison
        self.nc.clear_and_free_semaphores(list(self.sems.allocated().values()))
        self.nc.all_engine_barrier()

    def __exit__(self, *args):
        ret = super().__exit__(*args)
        _split_sync_waits(self.nc)
        return ret


def _build():
    nc = bass.Bass("TRN2")
    lab_d = nc.declare_dram_parameter(
        "labels", [ROWS_PER_CORE, N], i32, isOutput=False
    )
    out_d = nc.declare_dram_parameter(
        "output", [ROWS_PER_CORE, N], f32, isOutput=False
    )
    res_d = nc.declare_dram_parameter("res", [P, RB * NCH], f32, isOutput=True)

    with TileContextSplitDrain(nc) as tc:
        with (
            tc.tile_pool(name="const", bufs=1) as constp,
            tc.tile_pool(name="lab", bufs=1) as labp,
            tc.tile_pool(name="outf", bufs=1) as outfp,
            tc.tile_pool(name="obf", bufs=2) as obfp,
            tc.tile_pool(name="cum", bufs=2) as cump,
            tc.tile_pool(name="tt", bufs=2) as ttp,
            tc.tile_pool(name="ln", bufs=2) as lnp,
            tc.tile_pool(name="r16", bufs=2) as rbp,
            tc.tile_pool(name="rc", bufs=2) as rcp,
            tc.tile_pool(name="w", bufs=2) as wp,
            tc.tile_pool(name="wd", bufs=2) as wdp,
        ):
            k16 = constp.tile([P, N], i16, tag="k16")
            nc.gpsimd.iota(k16[:], pattern=[[1, N]], base=1, channel_multiplier=0)
            acc = constp.tile([P, RB * NCH], f32, tag="acc")
            zb = constp.tile([P, 1], f32, tag="zb")
            nc.gpsimd.memset(zb[:], 0.0)

            for rb in range(RB):
                labt = labp.tile([P, N], i32)
                outt = outfp.tile([P, N], f32)
                rows = slice(rb * P, (rb + 1) * P)
                nc.sync.dma_start(out=labt[:], in_=lab_d[rows, :])
                nc.sync.dma_start(out=outt[:], in_=out_d[rows, :])

                # cumsum along the row: state = lab + state  (op1=bypass)
                cum = cump.tile([P, N], bf16)
                nc.vector.tensor_tensor_scan(
                    cum[:], labt[:], labt[:], 0.0, op0=Alu.add, op1=Alu.bypass
                )

                # f32 -> bf16 cast of output on GPSIMD (1-input = line rate)
                obf = obfp.tile([P, N], bf16)
                for c in range(NCH):
                    sl = slice(c * W, (c + 1) * W)
                    nc.gpsimd.tensor_copy(obf[:, sl], outt[:, sl])

                # T = cum[:, -1] as f32 per-row scalar
                T = ttp.tile([P, 1], f32)
                nc.vector.tensor_copy(T[:], cum[:, N - 1 : N])

                for c in range(NCH):
                    sl = slice(c * W, (c + 1) * W)
                    # r = 1/(k+T) via exp(-ln(k+T)); both on ACT, bf16 out
                    lnd = lnp.tile([P, W], f32)
                    nc.scalar.activation(
                        lnd[:], k16[:, sl], Act.Ln, bias=T[:], scale=1.0
                    )
                    r16 = rbp.tile([P, W], bf16)
                    nc.scalar.activation(
                        r16[:], lnd[:], Act.Exp, bias=zb[:], scale=-1.0
                    )
                    rc = rcp.tile([P, W], bf16)
                    nc.vector.tensor_mul(rc[:], r16[:], cum[:, sl])
                    w = wp.tile([P, W], bf16)
                    nc.vector.tensor_mul(w[:], rc[:], obf[:, sl])
                    # scaled sum via ACT Copy + accum_out
                    wd = wdp.tile([P, W], bf16)
                    nc.scalar.activation(
                        wd[:],
                        w[:],
                        Act.Copy,
                        bias=0.0,
                        scale=-2.0 / B,
                        accum_out=acc[:, rb * NCH + c : rb * NCH + c + 1],
                    )

            nc.sync.dma_start(out=res_d[:], in_=acc[:])
    return nc


_NC = None


def kernel(output: np.ndarray, labels: np.ndarray) -> np.ndarray:
    global _NC
    if _NC is None:
        _NC = _build()

    out2 = np.ascontiguousarray(
        np.squeeze(np.asarray(output), axis=2), dtype=np.float32
    )
    lab2 = np.ascontiguousarray(np.asarray(labels), dtype=np.int32)

    in_maps = []
    for c in range(NCORES):
        rows = slice(c * ROWS_PER_CORE, (c + 1) * ROWS_PER_CORE)
        in_maps.append(
            {
                "output": np.ascontiguousarray(out2[rows]),
                "labels": np.ascontiguousarray(lab2[rows]),
            }
        )

    res = run_bass_kernel_spmd(_NC, in_maps, list(range(NCORES))).results
    total = np.float64(0.0)
    for r in res:
        total += np.float64(r["res"].sum(dtype=np.float64))
    return np.float32(total)


# revision 5
# speedup vs baseline: 5.4270x; 5.4270x over previous
"""Trainium2 Bass kernel for nn_ChoopyLoss (F1@k weighted loss).

Math: for each row i,
    cum[i,j] = prefix-sum of labels[i, :j+1]
    T[i]     = cum[i, N-1]  (total relevant)
    f1[i,j]  = 2*cum[i,j] / (j+1 + T[i])     # algebraic simplification of
                                             # 2*prec*rec/(prec+rec); holds
                                             # for cum==0 and T==0 too
    loss = -(1/B) * sum_{i,j} output[i,j] * f1[i,j]

Sharding: pure data parallel over batch. Each of the 8 cores gets 256 rows,
computes per-partition partial sums, host sums the 8x[128,16] partials.

Per-core pipeline (2 row-blocks of 128 rows x 8192):
  DMA  : labels(int32) + output(f32) full-row loads
  DVE  : tensor_tensor_scan (cumsum) -> bf16; reciprocal_approx_fast;
         bf16 tensor_mul; tensor_tensor_reduce (fused mul+sum, scale=-2/B)
  ACT  : f32->bf16 cast of output; d = k + T (iota int16 + per-row bias)
  POOL : f32->bf16 cast of reciprocals; iota (one-time)
"""

import numpy as np

import concourse.bass as bass
import concourse.mybir as mybir
from concourse.bass_utils import run_bass_kernel_spmd
from concourse.tile import TileContext
from concourse.vector_clock import ScopedClock

B, N = 2048, 8192
NCORES = 8
ROWS_PER_CORE = B // NCORES          # 256
P = 128
RB = ROWS_PER_CORE // P              # 2 row-blocks
W = 1024                             # chunk width for pointwise stages
NCH = N // W

f32 = mybir.dt.float32
bf16 = mybir.dt.bfloat16
i32 = mybir.dt.int32
i16 = mybir.dt.int16
Alu = mybir.AluOpType
Act = mybir.ActivationFunctionType


MAX_WAITS = 1  # this walrus build rejects >MAX_WAITS sync waits per instruction


def _split_sync_waits(nc, max_waits=MAX_WAITS):
    """Walrus here rejects instructions carrying many sem waits ("Too many
    sync wait commands"). Hoist excess waits onto same-engine NoOps placed
    immediately before the instruction."""
    import bass_rust

    for f in nc.m.functions:
        for bb in f.blocks:
            new_insts = []
            for inst in bb.instructions:
                si = inst.sync_info
                waits = list(si.on_wait) if si and si.on_wait else []
                if len(waits) > max_waits:
                    keep = waits[:max_waits]
                    extra = waits[max_waits:]
                    for i in range(0, len(extra), max_waits):
                        nop = bass_rust.InstNoOp(
                            name=nc.get_next_instruction_name(), ins=[], outs=[]
                        )
                        nop.engine = inst.engine
                        nop.sync_info = mybir.SyncInfo(
                            on_wait=extra[i : i + max_waits], on_update=[]
                        )
                        nc.register_instruction(nop, overwrite=True)
                        new_insts.append(nop)
                    si.on_wait = keep
                new_insts.append(inst)
            bb.instructions[:] = new_insts


class TileContextSplitDrain(TileContext):
    """Stock TileContext puts one wait per live proc on the kernel-tail
    drain; pre-split those, then run the generic wait-splitter on exit."""

    def _drain_and_barrier(self, tick_clock, wait_clock):
        nop = self.nc.sync.nop(nofuse=True, hint="pre_drain_waits")
        wait_clock.add_sem_waits(
            nop.ins, ScopedClock({None: tick_clock.global_clock})
        )
        si = nop.ins.sync_info
        waits = list(si.on_wait or []) if si else []
        if si:
            si.on_wait = waits[:1]
        for w in waits[1:]:
            n2 = self.nc.sync.nop(nofuse=True, hint="pre_drain_waits")
            n2.ins.sync_info = mybir.SyncInfo(on_wait=[w], on_update=[])

        self.nc.sync.drain()
        self.nc.all_engine_barrier()
        assert self.sems is not None
        popped = self.nc._tile_sem_poison_stack.pop()
        assert popped is self._sem_poison
        self.nc.clear_and_free_semaphores(list(self.sems.allocated().values()))
        self.nc.all_engine_barrier()

    def __exit__(self, *args):
        ret = super().__exit__(*args)
        _split_sync_waits(self.nc)
        return ret


def _build(repeat: int = 1):
    import contextlib

    nc = bass.Bass("TRN2")
    lab_d = nc.declare_dram_parameter(
        "labels", [ROWS_PER_CORE, N], i32, isOutput=False
    )
    out_d = nc.declare_dram_parameter(
        "output", [ROWS_PER_CORE, N], f32, isOutput=False
    )
    res_d = nc.declare_dram_parameter("res", [P, RB * NCH], f32, isOutput=True)

    with TileContextSplitDrain(nc) as tc:
        rep_ctx = (
            tc.For_i(0, repeat, 1) if repeat > 1 else contextlib.nullcontext()
        )
        with (
            rep_ctx,
            tc.tile_pool(name="const", bufs=1) as constp,
            tc.tile_pool(name="lab", bufs=1) as labp,
            tc.tile_pool(name="outf", bufs=1) as outfp,
            tc.tile_pool(name="obf", bufs=2) as obfp,
            tc.tile_pool(name="cum", bufs=2) as cump,
            tc.tile_pool(name="tt", bufs=2) as ttp,
            tc.tile_pool(name="ln", bufs=2) as lnp,
            tc.tile_pool(name="r16", bufs=2) as rbp,
            tc.tile_pool(name="rc", bufs=2) as rcp,
            tc.tile_pool(name="w", bufs=2) as wp,
            tc.tile_pool(name="wd", bufs=2) as wdp,
        ):
            k16 = constp.tile([P, N], i16, tag="k16")
            nc.gpsimd.iota(k16[:], pattern=[[1, N]], base=1, channel_multiplier=0)
            acc = constp.tile([P, RB * NCH], f32, tag="acc")
            zb = constp.tile([P, 1], f32, tag="zb")
            nc.gpsimd.memset(zb[:], 0.0)

            for rb in range(RB):
                labt = labp.tile([P, N], i32)
                outt = outfp.tile([P, N], f32)
                rows = slice(rb * P, (rb + 1) * P)
                nc.sync.dma_start(out=labt[:], in_=lab_d[rows, :])
                nc.sync.dma_start(out=outt[:], in_=out_d[rows, :])

                # cumsum along the row: state = lab + state  (op1=bypass)
                cum = cump.tile([P, N], bf16)
                nc.vector.tensor_tensor_scan(
                    cum[:], labt[:], labt[:], 0.0, op0=Alu.add, op1=Alu.bypass
                )

                # f32 -> bf16 cast of output on GPSIMD (1-input = line rate)
                obf = obfp.tile([P, N], bf16)
                for c in range(NCH):
                    sl = slice(c * W, (c + 1) * W)
                    nc.gpsimd.tensor_copy(obf[:, sl], outt[:, sl])

                # T = cum[:, -1] as f32 per-row scalar
                T = ttp.tile([P, 1], f32)
                nc.vector.tensor_copy(T[:], cum[:, N - 1 : N])

                for c in range(NCH):
                    sl = slice(c * W, (c + 1) * W)
                    # r = 1/(k+T) via exp(-ln(k+T)); both on ACT, bf16 out
                    lnd = lnp.tile([P, W], f32)
                    nc.scalar.activation(
                        lnd[:], k16[:, sl], Act.Ln, bias=T[:], scale=1.0
                    )
                    r16 = rbp.tile([P, W], bf16)
                    nc.scalar.activation(
                        r16[:], lnd[:], Act.Exp, bias=zb[:], scale=-1.0
                    )
                    rc = rcp.tile([P, W], bf16)
                    nc.vector.tensor_mul(rc[:], r16[:], cum[:, sl])
                    w = wp.tile([P, W], bf16)
                    nc.vector.tensor_mul(w[:], rc[:], obf[:, sl])
                    # scaled sum via ACT Copy + accum_out
                    wd = wdp.tile([P, W], bf16)
                    nc.scalar.activation(
                        wd[:],
                        w[:],
                        Act.Copy,
                        bias=0.0,
                        scale=-2.0 / B,
                        accum_out=acc[:, rb * NCH + c : rb * NCH + c + 1],
                    )

            nc.sync.dma_start(out=res_d[:], in_=acc[:])
    return nc


_NC = None


def kernel(output: np.ndarray, labels: np.ndarray) -> np.ndarray:
    global _NC
    if _NC is None:
        _NC = _build()

    out2 = np.ascontiguousarray(
        np.squeeze(np.asarray(output), axis=2), dtype=np.float32
    )
    lab2 = np.ascontiguousarray(np.asarray(labels), dtype=np.int32)

    in_maps = []
    for c in range(NCORES):
        rows = slice(c * ROWS_PER_CORE, (c + 1) * ROWS_PER_CORE)
        in_maps.append(
            {
                "output": np.ascontiguousarray(out2[rows]),
                "labels": np.ascontiguousarray(lab2[rows]),
            }
        )

    res = run_bass_kernel_spmd(_NC, in_maps, list(range(NCORES))).results
    total = np.float64(0.0)
    for r in res:
        total += np.float64(r["res"].sum(dtype=np.float64))
    return np.float32(total)


# revision 7
# speedup vs baseline: 6.8357x; 1.2596x over previous
"""Trainium2 Bass kernel for nn_ChoopyLoss (F1@k weighted loss).

Math: for each row i,
    cum[i,j] = prefix-sum of labels[i, :j+1]
    T[i]     = cum[i, N-1]  (total relevant)
    f1[i,j]  = 2*cum[i,j] / (j+1 + T[i])     # algebraic simplification of
                                             # 2*prec*rec/(prec+rec); holds
                                             # for cum==0 and T==0 too
    loss = -(1/B) * sum_{i,j} output[i,j] * f1[i,j]

Sharding: pure data parallel over batch. Each of the 8 cores gets 256 rows,
computes per-partition partial sums, host sums the 8x[128,16] partials.

Per-core pipeline (2 row-blocks of 128 rows x 8192):
  DMA  : labels(int32) + output(f32) full-row loads
  DVE  : tensor_tensor_scan (cumsum) -> bf16; reciprocal_approx_fast;
         bf16 tensor_mul; tensor_tensor_reduce (fused mul+sum, scale=-2/B)
  ACT  : f32->bf16 cast of output; d = k + T (iota int16 + per-row bias)
  POOL : f32->bf16 cast of reciprocals; iota (one-time)
"""

import numpy as np

import concourse.bass as bass
import concourse.mybir as mybir
from concourse.bass_utils import run_bass_kernel_spmd
from concourse.tile import TileContext
from concourse.vector_clock import ScopedClock

B, N = 2048, 8192
NCORES = 8
ROWS_PER_CORE = B // NCORES          # 256
P = 128
RB = ROWS_PER_CORE // P              # 2 row-blocks
W = 2048                             # chunk width for pointwise stages
NCH = N // W

f32 = mybir.dt.float32
bf16 = mybir.dt.bfloat16
i32 = mybir.dt.int32
i16 = mybir.dt.int16
Alu = mybir.AluOpType
Act = mybir.ActivationFunctionType


MAX_WAITS = 1  # this walrus build rejects >MAX_WAITS sync waits per instruction


def _split_sync_waits(nc, max_waits=MAX_WAITS):
    """Walrus here rejects instructions carrying many sem waits ("Too many
    sync wait commands"). Hoist excess waits onto same-engine NoOps placed
    immediately before the instruction."""
    import bass_rust

    for f in nc.m.functions:
        for bb in f.blocks:
            new_insts = []
            for inst in bb.instructions:
                si = inst.sync_info
                waits = list(si.on_wait) if si and si.on_wait else []
                if len(waits) > max_waits:
                    keep = waits[:max_waits]
                    extra = waits[max_waits:]
                    for i in range(0, len(extra), max_waits):
                        nop = bass_rust.InstNoOp(
                            name=nc.get_next_instruction_name(), ins=[], outs=[]
                        )
                        nop.engine = inst.engine
                        nop.sync_info = mybir.SyncInfo(
                            on_wait=extra[i : i + max_waits], on_update=[]
                        )
                        nc.register_instruction(nop, overwrite=True)
                        new_insts.append(nop)
                    si.on_wait = keep
                new_insts.append(inst)
            bb.instructions[:] = new_insts


class TileContextSplitDrain(TileContext):
    """Stock TileContext puts one wait per live proc on the kernel-tail
    drain; pre-split those, then run the generic wait-splitter on exit."""

    def _drain_and_barrier(self, tick_clock, wait_clock):
        nop = self.nc.sync.nop(nofuse=True, hint="pre_drain_waits")
        wait_clock.add_sem_waits(
            nop.ins, ScopedClock({None: tick_clock.global_clock})
        )
        si = nop.ins.sync_info
        waits = list(si.on_wait or []) if si else []
        if si:
            si.on_wait = waits[:1]
        for w in waits[1:]:
            n2 = self.nc.sync.nop(nofuse=True, hint="pre_drain_waits")
            n2.ins.sync_info = mybir.SyncInfo(on_wait=[w], on_update=[])

        self.nc.sync.drain()
        self.nc.all_engine_barrier()
        assert self.sems is not None
        popped = self.nc._tile_sem_poison_stack.pop()
        assert popped is self._sem_poison
        self.nc.clear_and_free_semaphores(list(self.sems.allocated().values()))
        self.nc.all_engine_barrier()

    def __exit__(self, *args):
        ret = super().__exit__(*args)
        _split_sync_waits(self.nc)
        return ret


def _build(repeat: int = 1):
    import contextlib

    nc = bass.Bass("TRN2")
    lab_d = nc.declare_dram_parameter(
        "labels", [ROWS_PER_CORE, N], i32, isOutput=False
    )
    out_d = nc.declare_dram_parameter(
        "output", [ROWS_PER_CORE, N], f32, isOutput=False
    )
    res_d = nc.declare_dram_parameter("res", [P, RB * NCH], f32, isOutput=True)

    with TileContextSplitDrain(nc) as tc:
        rep_ctx = (
            tc.For_i(0, repeat, 1) if repeat > 1 else contextlib.nullcontext()
        )
        with (
            rep_ctx,
            tc.tile_pool(name="const", bufs=1) as constp,
            tc.tile_pool(name="lab", bufs=1) as labp,
            tc.tile_pool(name="outf", bufs=1) as outfp,
            tc.tile_pool(name="cum", bufs=2) as cump,
            tc.tile_pool(name="tt", bufs=2) as ttp,
            tc.tile_pool(name="ln", bufs=2) as lnp,
            tc.tile_pool(name="r16", bufs=2) as rbp,
            tc.tile_pool(name="rc", bufs=2) as rcp,
            tc.tile_pool(name="w", bufs=2) as wp,
            tc.tile_pool(name="wd", bufs=2) as wdp,
        ):
            k16 = constp.tile([P, N], i16, tag="k16")
            nc.gpsimd.iota(k16[:], pattern=[[1, N]], base=1, channel_multiplier=0)
            acc = constp.tile([P, RB * NCH], f32, tag="acc")
            zb = constp.tile([P, 1], f32, tag="zb")
            nc.gpsimd.memset(zb[:], 0.0)

            for rb in range(RB):
                labt = labp.tile([P, N], i32)
                outt = outfp.tile([P, N], f32)
                rows = slice(rb * P, (rb + 1) * P)
                # split loads across DMA queues
                H = N // 2
                nc.sync.dma_start(out=labt[:, :H], in_=lab_d[rows, :H])
                nc.sync.dma_start(out=labt[:, H:], in_=lab_d[rows, H:])
                nc.sync.dma_start(out=outt[:, :H], in_=out_d[rows, :H])
                nc.sync.dma_start(out=outt[:, H:], in_=out_d[rows, H:])

                # cumsum along the row: state = lab + state  (op1=bypass)
                cum = cump.tile([P, N], bf16)
                nc.vector.tensor_tensor_scan(
                    cum[:], labt[:], labt[:], 0.0, op0=Alu.add, op1=Alu.bypass
                )

                # T = cum[:, -1] as f32 per-row scalar
                T = ttp.tile([P, 1], f32)
                nc.vector.tensor_copy(T[:], cum[:, N - 1 : N])

                for c in range(NCH):
                    sl = slice(c * W, (c + 1) * W)
                    # r = 1/(k+T) via exp(-ln(k+T)); both on ACT, bf16 out
                    lnd = lnp.tile([P, W], f32)
                    nc.scalar.activation(
                        lnd[:], k16[:, sl], Act.Ln, bias=T[:], scale=1.0
                    )
                    r16 = rbp.tile([P, W], bf16)
                    nc.scalar.activation(
                        r16[:], lnd[:], Act.Exp, bias=zb[:], scale=-1.0
                    )
                    rc = rcp.tile([P, W], bf16)
                    nc.vector.tensor_mul(rc[:], r16[:], cum[:, sl])
                    w = wp.tile([P, W], bf16)
                    nc.vector.tensor_mul(w[:], rc[:], outt[:, sl])
                    # scaled sum via ACT Copy + accum_out
                    wd = wdp.tile([P, W], bf16)
                    nc.scalar.activation(
                        wd[:],
                        w[:],
                        Act.Copy,
                        bias=0.0,
                        scale=-2.0 / B,
                        accum_out=acc[:, rb * NCH + c : rb * NCH + c + 1],
                    )

            nc.sync.dma_start(out=res_d[:], in_=acc[:])
    return nc


_NC = None


def kernel(output: np.ndarray, labels: np.ndarray) -> np.ndarray:
    global _NC
    if _NC is None:
        _NC = _build()

    out2 = np.ascontiguousarray(
        np.squeeze(np.asarray(output), axis=2), dtype=np.float32
    )
    lab2 = np.ascontiguousarray(np.asarray(labels), dtype=np.int32)

    in_maps = []
    for c in range(NCORES):
        rows = slice(c * ROWS_PER_CORE, (c + 1) * ROWS_PER_CORE)
        in_maps.append(
            {
                "output": np.ascontiguousarray(out2[rows]),
                "labels": np.ascontiguousarray(lab2[rows]),
            }
        )

    res = run_bass_kernel_spmd(_NC, in_maps, list(range(NCORES))).results
    total = np.float64(0.0)
    for r in res:
        total += np.float64(r["res"].sum(dtype=np.float64))
    return np.float32(total)


# revision 10
# speedup vs baseline: 6.9213x; 1.0125x over previous
"""Trainium2 Bass kernel for nn_ChoopyLoss (F1@k weighted loss).

Math: for each row i,
    cum[i,j] = prefix-sum of labels[i, :j+1]
    T[i]     = cum[i, N-1]  (total relevant)
    f1[i,j]  = 2*cum[i,j] / (j+1 + T[i])     # algebraic simplification of
                                             # 2*prec*rec/(prec+rec); holds
                                             # for cum==0 and T==0 too
    loss = -(1/B) * sum_{i,j} output[i,j] * f1[i,j]

Sharding: pure data parallel over batch. Each of the 8 cores gets 256 rows,
computes per-partition partial sums, host sums the 8x[128,16] partials.

Per-core pipeline (2 row-blocks of 128 rows x 8192):
  DMA  : labels(int32) + output(f32) full-row loads
  DVE  : tensor_tensor_scan (cumsum) -> bf16; reciprocal_approx_fast;
         bf16 tensor_mul; tensor_tensor_reduce (fused mul+sum, scale=-2/B)
  ACT  : f32->bf16 cast of output; d = k + T (iota int16 + per-row bias)
  POOL : f32->bf16 cast of reciprocals; iota (one-time)
"""

import numpy as np

import concourse.bass as bass
import concourse.mybir as mybir
from concourse.bass_utils import run_bass_kernel_spmd
from concourse.tile import TileContext
from concourse.vector_clock import ScopedClock

B, N = 2048, 8192
NCORES = 8
ROWS_PER_CORE = B // NCORES          # 256
P = 128
RB = ROWS_PER_CORE // P              # 2 row-blocks
W = 2048                             # chunk width for pointwise stages
NCH = N // W

f32 = mybir.dt.float32
bf16 = mybir.dt.bfloat16
i32 = mybir.dt.int32
i16 = mybir.dt.int16
Alu = mybir.AluOpType
Act = mybir.ActivationFunctionType


MAX_WAITS = 1  # this walrus build rejects >MAX_WAITS sync waits per instruction


def _split_sync_waits(nc, max_waits=MAX_WAITS):
    """Walrus here rejects instructions carrying many sem waits ("Too many
    sync wait commands"). Hoist excess waits onto same-engine NoOps placed
    immediately before the instruction."""
    import bass_rust

    for f in nc.m.functions:
        for bb in f.blocks:
            new_insts = []
            for inst in bb.instructions:
                si = inst.sync_info
                waits = list(si.on_wait) if si and si.on_wait else []
                if len(waits) > max_waits:
                    keep = waits[:max_waits]
                    extra = waits[max_waits:]
                    for i in range(0, len(extra), max_waits):
                        nop = bass_rust.InstNoOp(
                            name=nc.get_next_instruction_name(), ins=[], outs=[]
                        )
                        nop.engine = inst.engine
                        nop.sync_info = mybir.SyncInfo(
                            on_wait=extra[i : i + max_waits], on_update=[]
                        )
                        nc.register_instruction(nop, overwrite=True)
                        new_insts.append(nop)
                    si.on_wait = keep
                new_insts.append(inst)
            bb.instructions[:] = new_insts


class TileContextSplitDrain(TileContext):
    """Stock TileContext puts one wait per live proc on the kernel-tail
    drain; pre-split those, then run the generic wait-splitter on exit."""

    def _drain_and_barrier(self, tick_clock, wait_clock):
        nop = self.nc.sync.nop(nofuse=True, hint="pre_drain_waits")
        wait_clock.add_sem_waits(
            nop.ins, ScopedClock({None: tick_clock.global_clock})
        )
        si = nop.ins.sync_info
        waits = list(si.on_wait or []) if si else []
        if si:
            si.on_wait = waits[:1]
        for w in waits[1:]:
            n2 = self.nc.sync.nop(nofuse=True, hint="pre_drain_waits")
            n2.ins.sync_info = mybir.SyncInfo(on_wait=[w], on_update=[])

        self.nc.sync.drain()
        self.nc.all_engine_barrier()
        assert self.sems is not None
        popped = self.nc._tile_sem_poison_stack.pop()
        assert popped is self._sem_poison
        self.nc.clear_and_free_semaphores(list(self.sems.allocated().values()))
        self.nc.all_engine_barrier()

    def __exit__(self, *args):
        ret = super().__exit__(*args)
        _split_sync_waits(self.nc)
        return ret


def _build(repeat: int = 1):
    import contextlib

    nc = bass.Bass("TRN2")
    lab_d = nc.declare_dram_parameter(
        "labels", [ROWS_PER_CORE, N], i32, isOutput=False
    )
    out_d = nc.declare_dram_parameter(
        "output", [ROWS_PER_CORE, N], f32, isOutput=False
    )
    res_d = nc.declare_dram_parameter("res", [1, 1], f32, isOutput=True)

    WD = 512   # DVE sub-op width: op duration ~ the drain-free point
    MM = 512   # matmul free width (one PSUM bank)

    with TileContextSplitDrain(nc) as tc:
        rep_ctx = (
            tc.For_i(0, repeat, 1) if repeat > 1 else contextlib.nullcontext()
        )
        with (
            rep_ctx,
            tc.tile_pool(name="const", bufs=1) as constp,
            tc.tile_pool(name="lab", bufs=1) as labp,
            tc.tile_pool(name="outf", bufs=1) as outfp,
            tc.tile_pool(name="cum", bufs=2) as cump,
            tc.tile_pool(name="tt", bufs=2) as ttp,
            tc.tile_pool(name="ln", bufs=2) as lnp,
            tc.tile_pool(name="r16", bufs=2) as rbp,
            tc.tile_pool(name="ob", bufs=2) as obp,
            tc.tile_pool(name="rc", bufs=2) as rcp,
            tc.tile_pool(name="w", bufs=3) as wp,
            tc.tile_pool(name="fin", bufs=1) as finp,
            tc.tile_pool(name="ps", bufs=1, space="PSUM") as psp,
        ):
            k16 = constp.tile([P, N], i16, tag="k16")
            nc.gpsimd.iota(k16[:], pattern=[[1, N]], base=1, channel_multiplier=0)
            zb = constp.tile([P, 1], f32, tag="zb")
            nc.gpsimd.memset(zb[:], 0.0)
            ones = constp.tile([P, 1], bf16, tag="ones")
            nc.gpsimd.memset(ones[:], 1.0)

            ps = psp.tile([1, MM], f32)
            n_mm = RB * NCH * (W // MM)
            mm_i = 0

            for rb in range(RB):
                labt = labp.tile([P, N], i32)
                outt = outfp.tile([P, N], f32)
                rows = slice(rb * P, (rb + 1) * P)
                # split loads across DMA queues
                H = N // 2
                nc.sync.dma_start(out=labt[:, :H], in_=lab_d[rows, :H])
                nc.sync.dma_start(out=labt[:, H:], in_=lab_d[rows, H:])
                nc.sync.dma_start(out=outt[:, :H], in_=out_d[rows, :H])
                nc.sync.dma_start(out=outt[:, H:], in_=out_d[rows, H:])

                # cumsum along the row: state = lab + state  (op1=bypass)
                cum = cump.tile([P, N], bf16)
                nc.vector.tensor_tensor_scan(
                    cum[:], labt[:], labt[:], 0.0, op0=Alu.add, op1=Alu.bypass
                )

                # T = cum[:, -1] as f32 per-row scalar
                T = ttp.tile([P, 1], f32)
                nc.vector.tensor_copy(T[:], cum[:, N - 1 : N])

                for c in range(NCH):
                    sl = slice(c * W, (c + 1) * W)
                    # r = 1/(k+T) via exp(-ln(k+T)); both on ACT, bf16 out
                    lnd = lnp.tile([P, W], f32)
                    nc.scalar.activation(
                        lnd[:], k16[:, sl], Act.Ln, bias=T[:], scale=1.0
                    )
                    r16 = rbp.tile([P, W], bf16)
                    nc.scalar.activation(
                        r16[:], lnd[:], Act.Exp, bias=zb[:], scale=-1.0
                    )
                    # out chunk f32 -> bf16 on ACT
                    ob = obp.tile([P, W], bf16)
                    nc.scalar.copy(out=ob[:], in_=outt[:, sl])
                    rc = rcp.tile([P, W], bf16)
                    w = wp.tile([P, W], bf16)
                    for s in range(W // WD):
                        ss = slice(s * WD, (s + 1) * WD)
                        nc.vector.tensor_mul(rc[:, ss], r16[:, ss], cum[:, sl][:, ss])
                        nc.vector.tensor_mul(w[:, ss], rc[:, ss], ob[:, ss])
                    # global-sum via PE: ones^T @ w accumulated in PSUM
                    for s in range(W // MM):
                        ss = slice(s * MM, (s + 1) * MM)
                        nc.tensor.matmul(
                            ps[:],
                            ones[:],
                            w[:, ss],
                            start=(mm_i == 0),
                            stop=(mm_i == n_mm - 1),
                        )
                        mm_i += 1

            # collapse [1, MM] psum partials to a scalar, then DMA out
            fin = finp.tile([1, 1], f32, tag="fin")
            nc.vector.tensor_reduce(
                fin[:], ps[:], axis=mybir.AxisListType.X, op=Alu.add
            )
            nc.sync.dma_start(out=res_d[:], in_=fin[:])
    return nc


_NC = None


def kernel(output: np.ndarray, labels: np.ndarray) -> np.ndarray:
    global _NC
    if _NC is None:
        _NC = _build()

    out2 = np.ascontiguousarray(
        np.squeeze(np.asarray(output), axis=2), dtype=np.float32
    )
    lab2 = np.ascontiguousarray(np.asarray(labels), dtype=np.int32)

    in_maps = []
    for c in range(NCORES):
        rows = slice(c * ROWS_PER_CORE, (c + 1) * ROWS_PER_CORE)
        in_maps.append(
            {
                "output": np.ascontiguousarray(out2[rows]),
                "labels": np.ascontiguousarray(lab2[rows]),
            }
        )

    res = run_bass_kernel_spmd(_NC, in_maps, list(range(NCORES))).results
    total = np.float64(0.0)
    for r in res:
        total += np.float64(r["res"].sum(dtype=np.float64))
    return np.float32(total * (-2.0 / B))


# revision 13
# speedup vs baseline: 7.8385x; 1.1325x over previous
"""Trainium2 Bass kernel for nn_ChoopyLoss (F1@k weighted loss).

Math: for each row i,
    cum[i,j] = prefix-sum of labels[i, :j+1]
    T[i]     = cum[i, N-1]  (total relevant)
    f1[i,j]  = 2*cum[i,j] / (j+1 + T[i])     # algebraic simplification of
                                             # 2*prec*rec/(prec+rec); holds
                                             # for cum==0 and T==0 too
    loss = -(1/B) * sum_{i,j} output[i,j] * f1[i,j]

Sharding: pure data parallel over batch. Each of the 8 cores gets 256 rows,
computes per-partition partial sums, host sums the 8x[128,16] partials.

Per-core pipeline (2 row-blocks of 128 rows x 8192):
  DMA  : labels(int32) + output(f32) full-row loads
  DVE  : tensor_tensor_scan (cumsum) -> bf16; reciprocal_approx_fast;
         bf16 tensor_mul; tensor_tensor_reduce (fused mul+sum, scale=-2/B)
  ACT  : f32->bf16 cast of output; d = k + T (iota int16 + per-row bias)
  POOL : f32->bf16 cast of reciprocals; iota (one-time)
"""

import numpy as np

import concourse.bass as bass
import concourse.mybir as mybir
from concourse.bass_utils import run_bass_kernel_spmd
from concourse.tile import TileContext
from concourse.vector_clock import ScopedClock

B, N = 2048, 8192
NCORES = 8
ROWS_PER_CORE = B // NCORES          # 256
P = 128
RB = ROWS_PER_CORE // P              # 2 row-blocks
W = 2048                             # chunk width for pointwise stages
NCH = N // W

f32 = mybir.dt.float32
bf16 = mybir.dt.bfloat16
i32 = mybir.dt.int32
i16 = mybir.dt.int16
Alu = mybir.AluOpType
Act = mybir.ActivationFunctionType


MAX_WAITS = 1  # this walrus build rejects >MAX_WAITS sync waits per instruction


def _split_sync_waits(nc, max_waits=MAX_WAITS):
    """Walrus here rejects instructions carrying many sem waits ("Too many
    sync wait commands"). Hoist excess waits onto same-engine NoOps placed
    immediately before the instruction."""
    import bass_rust

    for f in nc.m.functions:
        for bb in f.blocks:
            new_insts = []
            for inst in bb.instructions:
                si = inst.sync_info
                waits = list(si.on_wait) if si and si.on_wait else []
                if len(waits) > max_waits:
                    keep = waits[:max_waits]
                    extra = waits[max_waits:]
                    for i in range(0, len(extra), max_waits):
                        nop = bass_rust.InstNoOp(
                            name=nc.get_next_instruction_name(), ins=[], outs=[]
                        )
                        nop.engine = inst.engine
                        nop.sync_info = mybir.SyncInfo(
                            on_wait=extra[i : i + max_waits], on_update=[]
                        )
                        nc.register_instruction(nop, overwrite=True)
                        new_insts.append(nop)
                    si.on_wait = keep
                new_insts.append(inst)
            bb.instructions[:] = new_insts


class TileContextSplitDrain(TileContext):
    """Stock TileContext puts one wait per live proc on the kernel-tail
    drain; pre-split those, then run the generic wait-splitter on exit."""

    def _drain_and_barrier(self, tick_clock, wait_clock):
        nop = self.nc.sync.nop(nofuse=True, hint="pre_drain_waits")
        wait_clock.add_sem_waits(
            nop.ins, ScopedClock({None: tick_clock.global_clock})
        )
        si = nop.ins.sync_info
        waits = list(si.on_wait or []) if si else []
        if si:
            si.on_wait = waits[:1]
        for w in waits[1:]:
            n2 = self.nc.sync.nop(nofuse=True, hint="pre_drain_waits")
            n2.ins.sync_info = mybir.SyncInfo(on_wait=[w], on_update=[])

        self.nc.sync.drain()
        self.nc.all_engine_barrier()
        assert self.sems is not None
        popped = self.nc._tile_sem_poison_stack.pop()
        assert popped is self._sem_poison
        self.nc.clear_and_free_semaphores(list(self.sems.allocated().values()))
        self.nc.all_engine_barrier()

    def __exit__(self, *args):
        ret = super().__exit__(*args)
        _split_sync_waits(self.nc)
        return ret


def _build(repeat: int = 1, strip: frozenset = frozenset()):
    import contextlib

    nc = bass.Bass("TRN2")
    lab_d = nc.declare_dram_parameter(
        "labels", [ROWS_PER_CORE, N], i32, isOutput=False
    )
    out_d = nc.declare_dram_parameter(
        "output", [ROWS_PER_CORE, N], f32, isOutput=False
    )
    res_d = nc.declare_dram_parameter("res", [1, 1], f32, isOutput=True)

    WD = 512   # DVE sub-op width: op duration ~ the drain-free point
    MM = 512   # matmul free width (one PSUM bank)

    with TileContextSplitDrain(nc) as tc:
        rep_ctx = (
            tc.For_i(0, repeat, 1) if repeat > 1 else contextlib.nullcontext()
        )
        with (
            rep_ctx,
            tc.tile_pool(name="const", bufs=1) as constp,
            tc.tile_pool(name="lab", bufs=4) as labp,
            tc.tile_pool(name="outf", bufs=4) as outfp,
            tc.tile_pool(name="cum", bufs=2) as cump,
            tc.tile_pool(name="ln", bufs=2) as lnp,
            tc.tile_pool(name="r16", bufs=2) as rbp,
            tc.tile_pool(name="ob", bufs=2) as obp,
            tc.tile_pool(name="rc", bufs=2) as rcp,
            tc.tile_pool(name="w", bufs=3) as wp,
            tc.tile_pool(name="fin", bufs=1) as finp,
            tc.tile_pool(name="ps", bufs=1, space="PSUM") as psp,
        ):
            k16 = constp.tile([P, N], i16, tag="k16")
            nc.gpsimd.iota(k16[:], pattern=[[1, N]], base=1, channel_multiplier=0)
            zb = constp.tile([P, 1], f32, tag="zb")
            nc.gpsimd.memset(zb[:], 0.0)
            ones = constp.tile([P, 1], bf16, tag="ones")
            nc.gpsimd.memset(ones[:], 1.0)

            ps = psp.tile([1, MM], f32)
            n_mm = RB * NCH * (W // MM)
            mm_i = 0

            for rb in range(RB):
                rows = slice(rb * P, (rb + 1) * P)
                # chunked loads: the scan chunk c starts as soon as lab
                # chunk c lands; everything pipelines under the DMA
                labts, outts = [], []
                for c in range(NCH):
                    sl = slice(c * W, (c + 1) * W)
                    labc = labp.tile([P, W], i32)
                    nc.sync.dma_start(out=labc[:], in_=lab_d[rows, sl])
                    labts.append(labc)
                for c in range(NCH):
                    sl = slice(c * W, (c + 1) * W)
                    outc = outfp.tile([P, W], f32)
                    nc.sync.dma_start(out=outc[:], in_=out_d[rows, sl])
                    outts.append(outc)

                # chained cumsum: state = lab + state (op1=bypass); the
                # carry rides as a bf16 scalar AP from the previous chunk
                cum = cump.tile([P, N], bf16)
                if "scan" not in strip:
                    for c in range(NCH):
                        sl = slice(c * W, (c + 1) * W)
                        init = 0.0 if c == 0 else cum[:, c * W - 1 : c * W]
                        nc.vector.tensor_tensor_scan(
                            cum[:, sl], labts[c][:], labts[c][:], init,
                            op0=Alu.add, op1=Alu.bypass,
                        )
                else:
                    nc.gpsimd.memset(cum[:, N - 1 : N], 1.0)

                # T = cum[:, -1] as f32 per-row scalar
                T = constp.tile([P, 1], f32, tag=f"T{rb}")
                nc.vector.tensor_copy(T[:], cum[:, N - 1 : N])

                for c in range(NCH):
                    sl = slice(c * W, (c + 1) * W)
                    # r = 1/(k+T) via exp(-ln(k+T)); both on ACT, bf16 out
                    lnd = lnp.tile([P, W], f32)
                    r16 = rbp.tile([P, W], bf16)
                    if "lnexp" not in strip:
                        nc.scalar.activation(
                            lnd[:], k16[:, sl], Act.Ln, bias=T[:], scale=1.0
                        )
                        nc.scalar.activation(
                            r16[:], lnd[:], Act.Exp, bias=zb[:], scale=-1.0
                        )
                    else:
                        nc.gpsimd.memset(r16[:, 0:1], 1.0)
                    # out chunk f32 -> bf16 on ACT
                    ob = obp.tile([P, W], bf16)
                    if "cast" not in strip:
                        nc.scalar.copy(out=ob[:], in_=outts[c][:])
                    else:
                        nc.gpsimd.memset(ob[:, 0:1], 1.0)
                    rc = rcp.tile([P, W], bf16)
                    w = wp.tile([P, W], bf16)
                    if "mul" not in strip:
                        for s in range(W // WD):
                            ss = slice(s * WD, (s + 1) * WD)
                            nc.vector.tensor_mul(rc[:, ss], r16[:, ss], cum[:, sl][:, ss])
                            nc.vector.tensor_mul(w[:, ss], rc[:, ss], ob[:, ss])
                    else:
                        nc.gpsimd.memset(w[:, 0:1], 1.0)
                    # global-sum via PE: ones^T @ w accumulated in PSUM
                    if "mm" not in strip:
                        for s in range(W // MM):
                            ss = slice(s * MM, (s + 1) * MM)
                            nc.tensor.matmul(
                                ps[:],
                                ones[:],
                                w[:, ss],
                                start=(mm_i == 0),
                                stop=(mm_i == n_mm - 1),
                            )
                            mm_i += 1
                    elif mm_i == 0:
                        nc.vector.memset(ps[:], 0.0)
                        mm_i += 1

            # collapse [1, MM] psum partials to a scalar, then DMA out
            fin = finp.tile([1, 1], f32, tag="fin")
            nc.vector.tensor_reduce(
                fin[:], ps[:], axis=mybir.AxisListType.X, op=Alu.add
            )
            nc.sync.dma_start(out=res_d[:], in_=fin[:])
    return nc


_NC = None


def kernel(output: np.ndarray, labels: np.ndarray) -> np.ndarray:
    global _NC
    if _NC is None:
        _NC = _build()

    out2 = np.ascontiguousarray(
        np.squeeze(np.asarray(output), axis=2), dtype=np.float32
    )
    lab2 = np.ascontiguousarray(np.asarray(labels), dtype=np.int32)

    in_maps = []
    for c in range(NCORES):
        rows = slice(c * ROWS_PER_CORE, (c + 1) * ROWS_PER_CORE)
        in_maps.append(
            {
                "output": np.ascontiguousarray(out2[rows]),
                "labels": np.ascontiguousarray(lab2[rows]),
            }
        )

    res = run_bass_kernel_spmd(_NC, in_maps, list(range(NCORES))).results
    total = np.float64(0.0)
    for r in res:
        total += np.float64(r["res"].sum(dtype=np.float64))
    return np.float32(total * (-2.0 / B))
